# revision 7
# baseline (speedup 1.0000x reference)
"""BlockCirculantConv on 8 Trainium2 NeuronCores.

The reference computes, per batch image b:
    xu = unfold(x[b])                       # (2304, 1024), f = c*9 + (di*3+dj)
    Y  = xu.flatten().reshape(1024, 2304)   # torch-faithful row-major reshape
    out_T = (Y @ W).T                       # W = expanded block-circulant (2304, 512)
    out[b] = out_T.reshape(512, 32, 32)
with W[q*64+s, p*64+t] = weight[p, q, (t-s) % 64]  (rfft product == circular conv).

Row n = 4c+j of Y is window j of channel c's flattened 9-shifted-image stack:
    Y[4c+j, k] = Z_c[j*2304 + k],   Z_c[dd*1024 + l] = xpad[c, l//32 + dd//3, l%32 + dd%3]
so out_T[:, 4c+j] = W^T @ Z_c-window-j: a dense (512x2304)@(2304x1024) GEMM per
image, columns enumerated as (j, c).

Device kernel per core (data-parallel over batch, 1 image/core):
  - HOST builds both GEMM operands in the exact SBUF tile layout,
    per-partition contiguous: sdram[p, kt, j, c] and wdram[p, kt, m], fp16.
    DMA is then a few large transfers with 2KB-per-partition descriptors
    (the previous 25-chunk strided plan produced 512B descriptors and
    ~700ns of HWDGE trigger serialization per chunk).
  - chunks are kt-ordered and sized small-first so the first matmul's gate
    data (kt0 of S and W) lands ~8us in; dummy matmuls on a zeroed tile
    bridge PE activity from ~6.6us so the HAM clock gate releases to
    2.4 GHz with no idle window re-throttling it.
  - 8 PSUM banks accumulate out_T as 4 m-tiles x 2 column-halves; kt 0..SPLIT-1
    round-robin all 8 banks (consumes chunks as they stream in), then each
    bank finishes kt SPLIT..17 alone so its drain + output DMA overlap the
    remaining matmuls instead of piling into the tail.
  - drain per (mt, nh): DVE/ACT copy PSUM fp32 -> SBUF fp16 (halves output
    DMA bytes; adds ~3e-4 rel err vs 2e-2 budget), sync-ring DMA out.
    Host permutes columns (j*256+c) -> n = 4c+j and upcasts to fp32.
"""

import sys

if "/opt/trn_rl_repo" not in sys.path:
    sys.path.insert(0, "/opt/trn_rl_repo")

import numpy as np

B, C, H, W_IMG = 8, 256, 32, 32
L = H * W_IMG               # 1024
BLK = 64
Q, P = 36, 8
K_FULL = Q * BLK            # 2304
M_OUT = P * BLK             # 512
KT = K_FULL // 128          # 18 k-tiles
N_CORES = 8

_CACHE = {}

N_PRE = 40                  # N=1 warmup matmuls on the const-0 AP (start ~6.8us)
N_WARM = 1                  # N=512 warmup matmuls on wz, bridging to first data
SPLIT = 10                  # kt phase boundary: round-robin -> per-psum finish
# kt group sizes for the S (sync ring) and W (scalar ring) input chunks,
# ascending kt; small first so the gate chunks land fast.
S_GROUPS = [1, 1, 2, 3, 4, 7]
W_GROUPS = [1, 2, 3, 5, 7]


def _build_nc():
    import concourse.bacc as bacc
    import concourse.tile as tile
    import concourse.mybir as mybir

    dt = mybir.dt
    din = dt.float16
    nc = bacc.Bacc("TRN2", target_bir_lowering=False, debug=False)

    sdram = nc.dram_tensor("sdram", [128, KT, 4, 256], din, kind="ExternalInput").ap()
    wdram = nc.dram_tensor("wdram", [128, KT, 512], din, kind="ExternalInput").ap()
    out = nc.dram_tensor("out", [M_OUT, L], din, kind="ExternalOutput").ap()

    f32 = dt.float32

    def groups_to_chunks(groups):
        chunks, kt0 = [], 0
        for n in groups:
            chunks.append((kt0, n))
            kt0 += n
        assert kt0 == KT
        return chunks

    schunks = groups_to_chunks(S_GROUPS)
    wchunks = groups_to_chunks(W_GROUPS)

    with tile.TileContext(nc) as tc:
        with (
            tc.tile_pool(name="wpool", bufs=1) as wpool,
            tc.tile_pool(name="spool", bufs=1) as spool,
            tc.tile_pool(name="opool", bufs=4) as opool,
            tc.tile_pool(name="ppool", bufs=1, space="PSUM") as ppool,
        ):
            wz = wpool.tile([128, 512], din, name="wz", tag="wz")
            nc.gpsimd.memset(wz[:], 0.0)

            psums = [
                ppool.tile([128, 512], f32, name=f"ps{i}", tag=f"ps{i}")
                for i in range(8)
            ]

            sbig = spool.tile([128, KT, 4, 256], din, name="sbig", tag="sbig")
            wbig = wpool.tile([128, KT, 512], din, name="wbig", tag="wbig")

            # PE warmup: HAM clock gate needs ~3.4us of sustained PE activity
            # to release 1.2 -> 2.4 GHz. Tiny const-AP matmuls start the
            # activity as soon as the engine preamble clears (~7.1us, before
            # wz's memset lands); wz dummies then bridge to first-data time.
            zap = nc.const_aps.aps[(f32, 0.0)]
            for _ in range(N_PRE):
                nc.tensor.matmul(psums[7][:1, :1], zap, zap, start=True, stop=True)
            for _ in range(N_WARM):
                nc.tensor.matmul(
                    psums[7][:], wz[:, :128], wz[:], start=True, stop=True
                )

            # Gate chunks first: kt0 of S (sync ring) and W (scalar ring).
            s0kt, s0n = schunks[0]
            w0kt, w0n = wchunks[0]
            nc.sync.dma_start(
                sbig[:, s0kt : s0kt + s0n, :, :], sdram[:, s0kt : s0kt + s0n, :, :]
            )
            nc.scalar.dma_start(
                wbig[:, w0kt : w0kt + w0n, :], wdram[:, w0kt : w0kt + w0n, :]
            )
            # Stall each HWDGE ring until both gate chunks have landed, so
            # the bulk transfers below don't steal HBM bandwidth from them
            # (unserialised, the gate lands ~2us later and the first real
            # matmul with it).
            r_sy = nc.alloc_register(mybir.EngineType.SP)
            r_sc = nc.alloc_register(mybir.EngineType.Activation)
            s_probe = sbig[0:1, 0, 0, 0:2].bitcast(dt.uint32)
            w_probe = wbig[0:1, 0, 0:2].bitcast(dt.uint32)
            nc.sync.reg_load(r_sy, s_probe)
            nc.sync.reg_load(r_sy, w_probe)
            nc.scalar.reg_load(r_sc, w_probe)
            nc.scalar.reg_load(r_sc, s_probe)

            # Bulk chunks, interleaved in ascending-kt order across rings.
            # tile_wait_until keeps the Tile list-scheduler from hoisting
            # these "ready" triggers above the gating reg_load waits in the
            # engine streams (the hardware stall comes from the reg_loads;
            # this only pins the order).
            with tc.tile_wait_until(0.006):
                si, wi = 1, 1
                while si < len(schunks) or wi < len(wchunks):
                    if si < len(schunks) and (
                        wi >= len(wchunks) or schunks[si][0] <= wchunks[wi][0]
                    ):
                        kt0, n = schunks[si]
                        nc.sync.dma_start(
                            sbig[:, kt0 : kt0 + n, :, :], sdram[:, kt0 : kt0 + n, :, :]
                        )
                        si += 1
                    else:
                        kt0, n = wchunks[wi]
                        nc.scalar.dma_start(
                            wbig[:, kt0 : kt0 + n, :], wdram[:, kt0 : kt0 + n, :]
                        )
                        wi += 1

            # Phase 1: kt 0..SPLIT-1 round-robin over all 8 psums (keeps the
            # PE fed from whichever chunks have landed).
            for kt in range(SPLIT):
                for mt in range(4):
                    for nh in range(2):
                        nc.tensor.matmul(
                            psums[mt * 2 + nh][:],
                            wbig[:, kt, mt * 128 : (mt + 1) * 128],
                            sbig[:, kt, nh * 2 : nh * 2 + 2, :],
                            start=(kt == 0),
                            stop=False,
                        )
            # Phase 2: finish one psum at a time; its drain + output DMA
            # overlap the remaining matmuls.
            for mt in range(4):
                for nh in range(2):
                    for kt in range(SPLIT, KT):
                        nc.tensor.matmul(
                            psums[mt * 2 + nh][:],
                            wbig[:, kt, mt * 128 : (mt + 1) * 128],
                            sbig[:, kt, nh * 2 : nh * 2 + 2, :],
                            start=False,
                            stop=(kt == KT - 1),
                        )
                    ot = opool.tile([128, 512], din, name="ot", tag="ot")
                    if nh == 0:
                        nc.vector.tensor_copy(ot[:], psums[mt * 2 + nh][:])
                    else:
                        nc.scalar.copy(ot[:], psums[mt * 2 + nh][:])
                    nc.sync.dma_start(
                        out[mt * 128 : (mt + 1) * 128, nh * 512 : (nh + 1) * 512],
                        ot[:],
                    )

    nc.compile()
    return nc


def _host_prep(x, weight):
    x = np.ascontiguousarray(x, dtype=np.float32)
    weight = np.ascontiguousarray(weight, dtype=np.float32)

    # Expanded block-circulant matrix: W[q*64+s, p*64+t] = weight[p, q, (t-s)%64]
    idx = (np.arange(BLK)[None, :] - np.arange(BLK)[:, None]) % BLK   # (s, t)
    w4 = weight[:, :, idx]                                            # (p, q, s, t)
    wmat = w4.transpose(1, 2, 0, 3).reshape(K_FULL, M_OUT)
    # wdram[p, kt, m] = wmat[kt*128+p, m]
    wdram = np.ascontiguousarray(
        wmat.reshape(KT, 128, M_OUT).transpose(1, 0, 2), dtype=np.float16
    )

    # Z[b, dd*1024 + l, c] = xpad[b, c, l//32 + dd//3, l%32 + dd%3]
    xp = np.pad(x, ((0, 0), (0, 0), (1, 1), (1, 1))).astype(np.float16)
    z = np.stack(
        [
            xp[:, :, di : di + 32, dj : dj + 32].reshape(B, C, L)
            for di in range(3)
            for dj in range(3)
        ],
        axis=1,
    )                                                   # (B, 9, C, 1024)
    z = z.transpose(0, 1, 3, 2).reshape(B, 9 * L, C)    # (B, 9216, C)
    # sdram[b, p, kt, j, c] = Z[b, j*2304 + kt*128 + p, c]
    sdram = np.ascontiguousarray(
        z.reshape(B, 4, KT, 128, C).transpose(0, 3, 2, 1, 4)
    )
    return sdram, wdram


def _run(x, weight, trace=False, trace_kwargs=None):
    from concourse.bass_utils import run_bass_kernel_spmd

    if "nc" not in _CACHE:
        _CACHE["nc"] = _build_nc()
    nc = _CACHE["nc"]

    sdram, wdram = _host_prep(x, weight)
    in_maps = [{"sdram": sdram[b], "wdram": wdram} for b in range(N_CORES)]
    res = run_bass_kernel_spmd(
        nc,
        in_maps,
        list(range(N_CORES)),
        trace=trace,
        **(trace_kwargs or {}),
    )
    out = np.stack([res.results[b]["out"] for b in range(N_CORES)])
    # device columns are (j*256 + c); output spatial index is n = 4c + j
    out = (
        out.astype(np.float32)
        .reshape(B, M_OUT, 4, 256)
        .transpose(0, 1, 3, 2)
        .reshape(B, M_OUT, H, W_IMG)
    )
    return np.ascontiguousarray(out), res


def kernel(x, weight):
    out, _ = _run(x, weight, trace=False)
    return out


# revision 9
# speedup vs baseline: 1.0060x; 1.0060x over previous
"""BlockCirculantConv on 8 Trainium2 NeuronCores.

The reference computes, per batch image b:
    xu = unfold(x[b])                       # (2304, 1024), f = c*9 + (di*3+dj)
    Y  = xu.flatten().reshape(1024, 2304)   # torch-faithful row-major reshape
    out_T = (Y @ W).T                       # W = expanded block-circulant (2304, 512)
    out[b] = out_T.reshape(512, 32, 32)
with W[q*64+s, p*64+t] = weight[p, q, (t-s) % 64]  (rfft product == circular conv).

Row n = 4c+j of Y is window j of channel c's flattened 9-shifted-image stack:
    Y[4c+j, k] = Z_c[j*2304 + k],   Z_c[dd*1024 + l] = xpad[c, l//32 + dd//3, l%32 + dd%3]
so out_T[:, 4c+j] = W^T @ Z_c-window-j: a dense (512x2304)@(2304x1024) GEMM per
image, columns enumerated as (j, c).

Device kernel per core (data-parallel over batch, 1 image/core):
  - HOST builds both GEMM operands in the exact SBUF tile layout,
    per-partition contiguous: sdram[p, kt, j, c] and wdram[p, kt, m], fp16.
    DMA is then a few large transfers with 2KB-per-partition descriptors
    (the previous 25-chunk strided plan produced 512B descriptors and
    ~700ns of HWDGE trigger serialization per chunk).
  - chunks are kt-ordered and sized small-first so the first matmul's gate
    data (kt0 of S and W) lands ~8us in; dummy matmuls on a zeroed tile
    bridge PE activity from ~6.6us so the HAM clock gate releases to
    2.4 GHz with no idle window re-throttling it.
  - 8 PSUM banks accumulate out_T as 4 m-tiles x 2 column-halves; kt 0..SPLIT-1
    round-robin all 8 banks (consumes chunks as they stream in), then each
    bank finishes kt SPLIT..17 alone so its drain + output DMA overlap the
    remaining matmuls instead of piling into the tail.
  - drain per (mt, nh): DVE/ACT copy PSUM fp32 -> SBUF fp16 (halves output
    DMA bytes; adds ~3e-4 rel err vs 2e-2 budget), sync-ring DMA out.
    Host permutes columns (j*256+c) -> n = 4c+j and upcasts to fp32.
"""

import sys

if "/opt/trn_rl_repo" not in sys.path:
    sys.path.insert(0, "/opt/trn_rl_repo")

import numpy as np

B, C, H, W_IMG = 8, 256, 32, 32
L = H * W_IMG               # 1024
BLK = 64
Q, P = 36, 8
K_FULL = Q * BLK            # 2304
M_OUT = P * BLK             # 512
KT = K_FULL // 128          # 18 k-tiles
N_CORES = 8

_CACHE = {}

N_PRE = 40                  # N=1 warmup matmuls on the const-0 AP (start ~6.8us)
N_WARM = 1                  # N=512 warmup matmuls on wz, bridging to first data
SPLIT = 10                  # kt phase boundary: round-robin -> per-psum finish
# kt group sizes for the S (sync ring) and W (scalar ring) input chunks,
# ascending kt; small first so the gate chunks land fast.
S_GROUPS = [1, 1, 2, 3, 4, 7]
W_GROUPS = [1, 2, 3, 5, 7]


def _build_nc():
    import concourse.bacc as bacc
    import concourse.tile as tile
    import concourse.mybir as mybir

    dt = mybir.dt
    din = dt.float16
    nc = bacc.Bacc("TRN2", target_bir_lowering=False, debug=False)

    sdram = nc.dram_tensor("sdram", [128, KT, 4, 256], din, kind="ExternalInput").ap()
    wdram = nc.dram_tensor("wdram", [128, KT, 512], din, kind="ExternalInput").ap()
    out = nc.dram_tensor("out", [M_OUT, L], din, kind="ExternalOutput").ap()

    f32 = dt.float32

    def groups_to_chunks(groups):
        chunks, kt0 = [], 0
        for n in groups:
            chunks.append((kt0, n))
            kt0 += n
        assert kt0 == KT
        return chunks

    schunks = groups_to_chunks(S_GROUPS)
    wchunks = groups_to_chunks(W_GROUPS)

    with tile.TileContext(nc) as tc:
        with (
            tc.tile_pool(name="wpool", bufs=1) as wpool,
            tc.tile_pool(name="spool", bufs=1) as spool,
            tc.tile_pool(name="opool", bufs=4) as opool,
            tc.tile_pool(name="ppool", bufs=1, space="PSUM") as ppool,
        ):
            wz = wpool.tile([128, 512], din, name="wz", tag="wz")
            nc.gpsimd.memset(wz[:], 0.0)

            psums = [
                ppool.tile([128, 512], f32, name=f"ps{i}", tag=f"ps{i}")
                for i in range(8)
            ]

            sbig = spool.tile([128, KT, 4, 256], din, name="sbig", tag="sbig")
            wbig = wpool.tile([128, KT, 512], din, name="wbig", tag="wbig")

            # PE warmup: HAM clock gate needs ~3.4us of sustained PE activity
            # to release 1.2 -> 2.4 GHz. Tiny const-AP matmuls start the
            # activity as soon as the engine preamble clears (~7.1us, before
            # wz's memset lands); wz dummies then bridge to first-data time.
            zap = nc.const_aps.aps[(f32, 0.0)]
            for _ in range(N_PRE):
                nc.tensor.matmul(psums[7][:1, :1], zap, zap, start=True, stop=True)
            for _ in range(N_WARM):
                nc.tensor.matmul(
                    psums[7][:], wz[:, :128], wz[:], start=True, stop=True
                )

            # Gate chunks: kt0 split by column-half on the sync ring (the
            # first 4 matmuls only need the nh0 half + W kt0, so the gate is
            # 131KB + 131KB landing in parallel on the two rings), then the
            # bulk in ascending-kt order. The gate is latency-bound
            # (~0.7us trigger + ~0.8us HWDGE start + transfer + ~0.4us sem
            # trickle), so smaller is faster; serializing bulk behind it
            # was tried and only moved the stalls downstream.
            nc.sync.dma_start(sbig[:, 0, 0:2, :], sdram[:, 0, 0:2, :])
            w0kt, w0n = wchunks[0]
            nc.scalar.dma_start(
                wbig[:, w0kt : w0kt + w0n, :], wdram[:, w0kt : w0kt + w0n, :]
            )
            nc.sync.dma_start(sbig[:, 0, 2:4, :], sdram[:, 0, 2:4, :])

            si, wi = 1, 1
            while si < len(schunks) or wi < len(wchunks):
                if si < len(schunks) and (
                    wi >= len(wchunks) or schunks[si][0] <= wchunks[wi][0]
                ):
                    kt0, n = schunks[si]
                    nc.sync.dma_start(
                        sbig[:, kt0 : kt0 + n, :, :], sdram[:, kt0 : kt0 + n, :, :]
                    )
                    si += 1
                else:
                    kt0, n = wchunks[wi]
                    nc.scalar.dma_start(
                        wbig[:, kt0 : kt0 + n, :], wdram[:, kt0 : kt0 + n, :]
                    )
                    wi += 1

            # Phase 1: kt 0..SPLIT-1 round-robin over all 8 psums, nh-major
            # so kt0's first 4 matmuls need only the nh0 gate half.
            for kt in range(SPLIT):
                for nh in range(2):
                    for mt in range(4):
                        nc.tensor.matmul(
                            psums[mt * 2 + nh][:],
                            wbig[:, kt, mt * 128 : (mt + 1) * 128],
                            sbig[:, kt, nh * 2 : nh * 2 + 2, :],
                            start=(kt == 0),
                            stop=False,
                        )
            # Phase 2: finish one psum at a time; its drain + output DMA
            # overlap the remaining matmuls.
            for mt in range(4):
                for nh in range(2):
                    for kt in range(SPLIT, KT):
                        nc.tensor.matmul(
                            psums[mt * 2 + nh][:],
                            wbig[:, kt, mt * 128 : (mt + 1) * 128],
                            sbig[:, kt, nh * 2 : nh * 2 + 2, :],
                            start=False,
                            stop=(kt == KT - 1),
                        )
                    ot = opool.tile([128, 512], din, name="ot", tag="ot")
                    if nh == 0:
                        nc.vector.tensor_copy(ot[:], psums[mt * 2 + nh][:])
                    else:
                        nc.scalar.copy(ot[:], psums[mt * 2 + nh][:])
                    nc.sync.dma_start(
                        out[mt * 128 : (mt + 1) * 128, nh * 512 : (nh + 1) * 512],
                        ot[:],
                    )

    nc.compile()
    return nc


def _host_prep(x, weight):
    x = np.ascontiguousarray(x, dtype=np.float32)
    weight = np.ascontiguousarray(weight, dtype=np.float32)

    # Expanded block-circulant matrix: W[q*64+s, p*64+t] = weight[p, q, (t-s)%64]
    idx = (np.arange(BLK)[None, :] - np.arange(BLK)[:, None]) % BLK   # (s, t)
    w4 = weight[:, :, idx]                                            # (p, q, s, t)
    wmat = w4.transpose(1, 2, 0, 3).reshape(K_FULL, M_OUT)
    # wdram[p, kt, m] = wmat[kt*128+p, m]
    wdram = np.ascontiguousarray(
        wmat.reshape(KT, 128, M_OUT).transpose(1, 0, 2), dtype=np.float16
    )

    # Z[b, dd*1024 + l, c] = xpad[b, c, l//32 + dd//3, l%32 + dd%3]
    xp = np.pad(x, ((0, 0), (0, 0), (1, 1), (1, 1))).astype(np.float16)
    z = np.stack(
        [
            xp[:, :, di : di + 32, dj : dj + 32].reshape(B, C, L)
            for di in range(3)
            for dj in range(3)
        ],
        axis=1,
    )                                                   # (B, 9, C, 1024)
    z = z.transpose(0, 1, 3, 2).reshape(B, 9 * L, C)    # (B, 9216, C)
    # sdram[b, p, kt, j, c] = Z[b, j*2304 + kt*128 + p, c]
    sdram = np.ascontiguousarray(
        z.reshape(B, 4, KT, 128, C).transpose(0, 3, 2, 1, 4)
    )
    return sdram, wdram


def _run(x, weight, trace=False, trace_kwargs=None):
    from concourse.bass_utils import run_bass_kernel_spmd

    if "nc" not in _CACHE:
        _CACHE["nc"] = _build_nc()
    nc = _CACHE["nc"]

    sdram, wdram = _host_prep(x, weight)
    in_maps = [{"sdram": sdram[b], "wdram": wdram} for b in range(N_CORES)]
    res = run_bass_kernel_spmd(
        nc,
        in_maps,
        list(range(N_CORES)),
        trace=trace,
        **(trace_kwargs or {}),
    )
    out = np.stack([res.results[b]["out"] for b in range(N_CORES)])
    # device columns are (j*256 + c); output spatial index is n = 4c + j
    out = (
        out.astype(np.float32)
        .reshape(B, M_OUT, 4, 256)
        .transpose(0, 1, 3, 2)
        .reshape(B, M_OUT, H, W_IMG)
    )
    return np.ascontiguousarray(out), res


def kernel(x, weight):
    out, _ = _run(x, weight, trace=False)
    return out
